# revision 31
# baseline (speedup 1.0000x reference)
"""AFT-Full on 8 TRN2 cores, v10: fp8-DR K/Q, fp16 V, streamed PE.

Same d-split pair sharding as v3 (core c: batch c//2, parity c%2; each
core projects K/V/Q for its d-half, mixes over all t, pairs exchange
activation halves, each core output-projects its own t-half).

Numerics: the AFT ratio is a weighted average of V over all 2048
positions, so its RELATIVE error equals the per-entry relative noise of
both the V values and the eK weights — V and the S_K/S_u sums must stay
>= fp16 precision (fp8 there costs ~2.5% output error). K tolerates fp8
(its 1.4% noise lands once, total ~9e-3 vs the 2e-2 gate): it runs fp8
DoubleRow from host-quantized x8/wk8 (x2048, undone by the exp's input
scale). Q stays fp8 DR (sigmoid output); everything formerly bf16 is
fp16 (same PE speed, 8x less rounding noise — HW error == numpy-sim
error). u8/eK8 stay fp8e4 for the E-side mixing only (2% of magnitude).

Scheduling (all measured on HW):
- Every 16-bit matmul streams at ~215ns/512-col slot; fp8 DR doubles
  the contraction per slot. The PE runs >98% busy between the fixed
  ~10us framework preamble and a ~2us output drain.
- DMA queues: gpsimd ring ~170GB/s, scalar ~75, sync ~60; transfers
  move full 2-16KB per-partition lines. A dma_start push BLOCKS its
  engine until the queue drains to ~2 in flight, so the scalar engine
  (which must run the exp/copy evacuations) carries only 3 early
  pushes, and wo/bo load late into the x pool's freed space.
- s-tile group 1 of K runs st-major and V group 1 k-major with
  interleaved evacuation, so PSUM banks recycle one at a time instead
  of stalling the PE at group boundaries.
- The S_u adds pair-sum on the DVE (fp16) before an 8-deep Pool-engine
  chain (a 16-deep Pool chain at 1.16us/add throttled ub recycling and
  stalled Q by ~9us); eK8 = fp8(eKb) is a DVE tensor_copy.
- All four E8 chunks stay resident (no WAR restage), so the gpsimd
  ring reaches the AllGather triggers immediately; b_in halves export
  as soon as their d-tiles finish; sync stays clean of bulk traffic so
  the exchange fires the moment data is ready (out-projection wave 2
  overlaps the pair exchange).
- (rat + bv) * sigQ fused into one DVE scalar_tensor_tensor; scol's
  partition-reduce hides behind the first mixing unit's matmuls; y is
  written fp16 per 512-column half, alternating sync/scalar queues.

Bias identities as in v3: bk cancels in the ratio, bv is a post-ratio
add, bq fuses into the sigmoid, bo rides the output-projection PSUM
evacuation.
"""

import sys

if "/opt/trn_rl_repo" not in sys.path:
    sys.path.insert(0, "/opt/trn_rl_repo")

import numpy as np
import ml_dtypes

F16 = np.float16
F8E4 = ml_dtypes.float8_e4m3

B, T, D = 4, 2048, 1024
TH = T // 2   # own-t rows per core
DH = D // 2   # d-half
P = 128
CH = 512
KT = D // P    # 8 k-tiles (full-d contractions)
DHT = DH // P  # 4 d-tiles in my half
ST = T // P    # 16 s-tiles
TC = T // CH   # 4 t-chunks of the full sequence
ESC = 4096.0   # fp8 scale on E; cancels in the ratio
KSC = 2048.0   # fp8 scale on wk; undone by the exp's input scale
QSC = 64.0     # fp8 scale on wq; undone by the sigmoid's input scale
WARM = 15

_cache = {}


def _build_nc():
    from contextlib import ExitStack

    import concourse.mybir as mybir
    import concourse.tile as tile
    from concourse import bacc
    from concourse.bass import ds

    dt = mybir.dt
    BF = dt.float16
    F32 = dt.float32
    F8 = dt.float8e4
    Act = mybir.ActivationFunctionType
    Alu = mybir.AluOpType
    DR = mybir.MatmulPerfMode.DoubleRow
    PAIRS = [[0, 1], [2, 3], [4, 5], [6, 7]]

    nc = bacc.Bacc("TRN2")

    # weights arrive host-restaged as SBUF images (row p = that
    # partition's full free-axis line) so every DMA moves 4-8KB lines
    x8F = nc.dram_tensor("x8F", [D, T], F8, kind="ExternalInput")
    xbF = nc.dram_tensor("xbF", [D, T], BF, kind="ExternalInput")
    wqF = nc.dram_tensor("wqF", [P, KT * DH], F8, kind="ExternalInput")
    wkF = nc.dram_tensor("wkF", [P, KT * DH], F8, kind="ExternalInput")
    wvF = nc.dram_tensor("wvF", [P, KT * DH], BF, kind="ExternalInput")
    woF = nc.dram_tensor("woF", [P, KT * D], BF, kind="ExternalInput")
    # E8 chunks pre-rotated by this core's parity: row j*128+p = the
    # SBUF line of partition p for AFT chunk j
    e8F = nc.dram_tensor("e8F", [TC * P, ST * CH], F8, kind="ExternalInput")
    bqc = nc.dram_tensor("bqc", [P, DHT], F32, kind="ExternalInput")
    bvc = nc.dram_tensor("bvc", [P, DHT], F32, kind="ExternalInput")
    bob = nc.dram_tensor("bob", [P, D], F32, kind="ExternalInput")
    y = nc.dram_tensor("y", [TH, D], BF, kind="ExternalOutput")

    x8_v = x8F.rearrange("(o p) t -> p o t", p=P)
    xb_v = xbF.rearrange("(o p) t -> p o t", p=P)
    e8_v = e8F.rearrange("(j p) t -> p j t", p=P)
    y_v = y.rearrange("(o p) e -> p o e", p=P)

    with tile.TileContext(nc) as tc:
        with (
            tc.tile_pool(name="big", bufs=1) as big,
            tc.tile_pool(name="w", bufs=2) as wpool,
            tc.tile_pool(name="tmp", bufs=6) as tmp,
            tc.tile_pool(name="bias", bufs=1) as biasp,
            tc.tile_pool(name="ew", bufs=4) as ewpool,
            tc.tile_pool(name="sg", bufs=1) as sgpool,
            tc.tile_pool(name="dram", bufs=4, space="DRAM") as dram,
            tc.tile_pool(name="psum", bufs=8, space="PSUM") as psum,
            ExitStack() as late_ctx,
        ):
            pid = nc.partition_id()
            par = pid % 2

            eKb = big.tile([P, ST, DH], BF, tag="eKb")
            eK8 = big.tile([P, ST, DH], F8, tag="eK8")
            u8 = big.tile([P, ST, DH], F8, tag="u8")
            sigQT = big.tile([P, DHT, T], BF, tag="sigQT")
            accK = big.tile([P, DH], F32, tag="accK")
            accU = big.tile([P, DH], F32, tag="accU")
            # scol columns: 0..3 = 4096*S_u per d-tile, 4..7 = 4096*S_K
            scol = big.tile([P, 2 * DHT], F32, tag="scol")
            ones1 = big.tile([P, 1], F32, tag="ones1")
            # AFT output in rotated-t order: chunk j holds t columns
            # (j*512 + p*1024) mod 2048 .. +512; j=0,1 own-t, j=2,3 pair-t
            oPC = [
                big.tile([P, DHT, CH], BF, tag=f"oPC{j}", name=f"oPC{j}")
                for j in range(TC)
            ]
            # staged E8 chunks, all four alive (processing order 2,3,0,1)
            E8c = {}

            nc.vector.memset(accK[:], 0.0)
            nc.vector.memset(accU[:], 0.0)
            nc.vector.memset(ones1[:], ESC)

            with tc.tile_pool(name="x", bufs=1) as xpool:
                x8 = xpool.tile([P, KT, T], F8, tag="x8")
                xb = xpool.tile([P, KT, T], BF, tag="xb")
                wk_s = wpool.tile([P, KT, DH], F8, tag="w")
                wv_s = wpool.tile([P, KT, DH], BF, tag="w")

                # PE warm-up during the input-DMA wait
                warm = biasp.tile([P, CH], BF, tag="warm")
                nc.vector.memset(warm[:], 0.0)
                pwarm = psum.tile([P, CH], F32, tag="ps", name="pwarm")
                for _ in range(WARM):
                    nc.tensor.matmul(
                        pwarm[:], warm[:, :P], warm[:], start=True, stop=True
                    )

                # DMA scheduling facts (measured): per-queue rates are
                # ~170GB/s on gpsimd's software ring, ~75 on scalar's,
                # ~60 on sync's; a push BLOCKS its issuing engine until
                # the queue has slots (~2 in flight), so the scalar
                # engine — which must run the K evacuation exps from
                # ~20us — gets at most 3 early pushes. Everything moves
                # full rows (2-16KB lines).
                nc.scalar.dma_start(x8[:, 0, :], x8_v[:, 0, :])
                nc.scalar.dma_start(x8[:, 4, :], x8_v[:, 4, :])
                nc.scalar.dma_start(x8[:, 6, :], x8_v[:, 6, :])
                nc.scalar.dma_start(xb[:, 1, :TH], xb_v[:, 1, :TH])
                nc.sync.dma_start(x8[:, 2, :], x8_v[:, 2, :])
                nc.gpsimd.dma_start(wk_s[:, :, :], wkF[:, :])
                nc.gpsimd.dma_start(x8[:, 1, :], x8_v[:, 1, :])
                nc.gpsimd.dma_start(x8[:, 3, :], x8_v[:, 3, :])
                nc.gpsimd.dma_start(x8[:, 5, :], x8_v[:, 5, :])
                nc.gpsimd.dma_start(x8[:, 7, :], x8_v[:, 7, :])
                nc.gpsimd.dma_start(xb[:, 0, :TH], xb_v[:, 0, :TH])
                nc.gpsimd.dma_start(wv_s[:, :4, :], wvF[:, : 4 * DH])
                nc.gpsimd.dma_start(wv_s[:, 4:, :], wvF[:, 4 * DH :])
                for k in [2, 3]:
                    nc.gpsimd.dma_start(xb[:, k, :TH], xb_v[:, k, :TH])
                for k in [4, 5, 6, 7]:
                    nc.sync.dma_start(xb[:, k, :TH], xb_v[:, k, :TH])
                # second halves all on gpsimd: V group 1 is k-major and
                # consumes them progressively from ~0.3 in
                for k in range(KT):
                    nc.gpsimd.dma_start(xb[:, k, TH:], xb_v[:, k, TH:])
                bq_s = biasp.tile([P, DHT], F32, tag="bq")
                nc.sync.dma_start(bq_s[:], bqc[:])
                bv_s = biasp.tile([P, DHT], F32, tag="bv")
                nc.sync.dma_start(bv_s[:], bvc[:])
                wq_s = wpool.tile([P, KT, DH], F8, tag="wq", bufs=1)
                nc.gpsimd.dma_start(wq_s[:, :, :], wqF[:, :])

                # E8 chunks (host pre-rotated by parity, static APs): all
                # FOUR buffers resident (no restage WAR), so the gpsimd
                # ring flows straight through to the collective triggers
                # — the first AllGather fires the moment its export
                # lands instead of queuing behind a gated restage.
                for j in [2, 3, 0, 1]:
                    ec = ewpool.tile([P, ST, CH], F8, tag="ewc", name=f"E8c{j}")
                    nc.gpsimd.dma_start(ec[:, :, :], e8_v[:, j, :])
                    E8c[j] = ec

                def k_evac(st):
                    nc.scalar.activation(
                        eKb[:, st, :], pks[st][:], Act.Exp, scale=1.0 / KSC
                    )
                    nc.vector.tensor_copy(eK8[:, st, :], eKb[:, st, :])
                    nc.gpsimd.tensor_tensor(
                        accK[:], accK[:], eKb[:, st, :], Alu.add
                    )

                # ---- K projection (d-half, fp8 DR) -> eKb/eK8/accK ----
                # group 0 contraction-major (overlaps the x8/wk8 DMA
                # ramp), group 1 st-major (PSUM banks recycle one at a
                # time through group 0's evacuations)
                pks = {}
                for st in range(8):
                    pks[st] = psum.tile([P, CH], F32, tag="ps", name=f"pk{st}")
                for kp in range(KT // 2):
                    ksl = slice(2 * kp, 2 * kp + 2)
                    for st in range(8):
                        nc.tensor.matmul(
                            pks[st][:],
                            x8[:, ksl, st * P : (st + 1) * P],
                            wk_s[:, ksl, :],
                            start=(kp == 0), stop=(kp == KT // 2 - 1),
                            perf_mode=DR,
                        )
                for st in range(8):
                    k_evac(st)
                for st in range(8, 16):
                    pks[st] = psum.tile([P, CH], F32, tag="ps", name=f"pk{st}")
                    for kp in range(KT // 2):
                        ksl = slice(2 * kp, 2 * kp + 2)
                        nc.tensor.matmul(
                            pks[st][:],
                            x8[:, ksl, st * P : (st + 1) * P],
                            wk_s[:, ksl, :],
                            start=(kp == 0), stop=(kp == KT // 2 - 1),
                            perf_mode=DR,
                        )
                    k_evac(st)

                # the Pool engine runs ~1.16us per [P,CH] add — a 16-deep
                # accU chain there throttles ub recycling and stalls the
                # PE into Q. Instead: DVE sums st-pairs in fp16 (fast,
                # 2x rate), gpsimd chains only the 8 pair sums.
                vodd = [None]

                def v_evac(st):
                    ub = tmp.tile([P, CH], BF, tag="ub", bufs=4)
                    nc.vector.tensor_tensor(
                        ub[:], eKb[:, st, :], pvs[st][:], Alu.mult
                    )
                    nc.scalar.copy(u8[:, st, :], ub[:])
                    if st % 2 == 0:
                        vodd[0] = ub
                    else:
                        up = tmp.tile([P, CH], BF, tag="up", bufs=3)
                        nc.vector.tensor_tensor(
                            up[:], vodd[0][:], ub[:], Alu.add
                        )
                        nc.gpsimd.tensor_tensor(
                            accU[:], accU[:], up[:], Alu.add
                        )

                # ---- V projection (d-half, bf16) -> u8/accU ----
                pvs = {}
                for st in range(8):
                    pvs[st] = psum.tile([P, CH], F32, tag="ps", name=f"pv{st}")
                for k in range(KT):
                    for st in range(8):
                        nc.tensor.matmul(
                            pvs[st][:],
                            xb[:, k, st * P : (st + 1) * P],
                            wv_s[:, k, :],
                            start=(k == 0), stop=(k == KT - 1),
                        )
                for st in range(8):
                    v_evac(st)
                # group 1 k-major (the xb second halves stream in per
                # k-tile during group 0), evacuations interleaved into
                # the last k pass so the banks recycle one at a time
                for st in range(8, 16):
                    pvs[st] = psum.tile([P, CH], F32, tag="ps", name=f"pv{st}")
                for k in range(KT):
                    for st in range(8, 16):
                        nc.tensor.matmul(
                            pvs[st][:],
                            xb[:, k, st * P : (st + 1) * P],
                            wv_s[:, k, :],
                            start=(k == 0), stop=(k == KT - 1),
                        )
                        if k == KT - 1:
                            v_evac(st)

                # ---- Q^T projection (d-half e, ALL t), fp8 DR -> sigQT ----
                for et in range(DHT):
                    esl = slice(et * P, (et + 1) * P)
                    for c in range(TC):
                        tsl = slice(c * CH, (c + 1) * CH)
                        pq = psum.tile([P, CH], F32, tag="ps")
                        for kp in range(KT // 2):
                            ksl = slice(2 * kp, 2 * kp + 2)
                            nc.tensor.matmul(
                                pq[:], wq_s[:, ksl, esl], x8[:, ksl, tsl],
                                start=(kp == 0), stop=(kp == KT // 2 - 1),
                                perf_mode=DR,
                            )
                        nc.scalar.activation(
                            sigQT[:, et, tsl], pq[:], Act.Sigmoid,
                            bias=bq_s[:, et : et + 1], scale=1.0 / QSC,
                        )

                # (scol is reduced inside the first mixing unit below, so
                # the PE never head-of-line waits on the acc chains)

            # wo + bo load into the space the x pool just freed, on the
            # scalar queue (ring free after the xb halves; keeps sync
            # CLEAN so the b_in exports below never queue behind a 2MB
            # transfer — that jitter stalled the out-projection ~10us)
            wopool = late_ctx.enter_context(tc.tile_pool(name="wop", bufs=1))
            wo_s = wopool.tile([P, KT, D], BF, tag="wo", name="wo_s")
            nc.scalar.dma_start(wo_s[:, :4, :], woF[:, : 4 * D])
            nc.scalar.dma_start(wo_s[:, 4:, :], woF[:, 4 * D :])
            bo_s = wopool.tile([P, D], F32, tag="bo")
            nc.scalar.dma_start(bo_s[:], bob[:])

            # ---- AFT mixing, fp8 DoubleRow over chunk pairs; pair-t
            # chunks (2,3) first so the exchange hides under (0,1) ----
            b_in = [None, None]
            S_out = [None, None]
            S_sb = [None, None]
            for i in range(2):
                b_in[i] = dram.tile([DH, CH], BF, name=f"bin{i}")
                S_out[i] = dram.tile([2 * DH, CH], BF, name=f"sout{i}")
                S_sb[i] = sgpool.tile(
                    [P, DHT, CH], BF, tag=f"ssb{i}", name=f"ssb{i}"
                )

            toffs = {}
            for j in range(TC):
                toffs[j] = nc.s_assert_within(
                    (j * CH + par * TH) & (T - 1), 0, T - CH,
                    skip_runtime_assert=True,
                )

            for pos, j in enumerate([2, 3, 0, 1]):
                for dti in range(DHT):
                    dsl = slice(dti * P, (dti + 1) * P)
                    pn = psum.tile([P, CH], F32, tag="ps", name="pn")
                    pd = psum.tile([P, CH], F32, tag="ps", name="pd")
                    for sp in range(ST // 2):
                        ksl = slice(2 * sp, 2 * sp + 2)
                        nc.tensor.matmul(
                            pn[:], u8[:, ksl, dsl], E8c[j][:, ksl, :],
                            start=(sp == 0), stop=(sp == ST // 2 - 1),
                            perf_mode=DR,
                        )
                    for sp in range(ST // 2):
                        ksl = slice(2 * sp, 2 * sp + 2)
                        nc.tensor.matmul(
                            pd[:], eK8[:, ksl, dsl], E8c[j][:, ksl, :],
                            start=(sp == 0), stop=(sp == ST // 2 - 1),
                            perf_mode=DR,
                        )
                    if pos == 0 and dti == 0:
                        # partition-reduce the column sums (scol = 4096*S)
                        # behind the first unit's matmuls: the tden/tnum
                        # evacuations below are the first scol readers
                        pscol = psum.tile(
                            [P, 2 * DHT], F32, tag="ps", name="pscol"
                        )
                        for dtc in range(DHT):
                            dcl = slice(dtc * P, (dtc + 1) * P)
                            nc.tensor.matmul(
                                pscol[:, dtc : dtc + 1], accU[:, dcl],
                                ones1[:], start=True, stop=True,
                            )
                            nc.tensor.matmul(
                                pscol[:, DHT + dtc : DHT + dtc + 1],
                                accK[:, dcl], ones1[:],
                                start=True, stop=True,
                            )
                        nc.scalar.copy(scol[:], pscol[:])
                    tden = tmp.tile([P, CH], F32, tag="rec")
                    nc.scalar.activation(
                        tden[:], pd[:], Act.Identity,
                        bias=scol[:, DHT + dti : DHT + dti + 1],
                    )
                    rec = tmp.tile([P, CH], F32, tag="rec")
                    nc.vector.reciprocal_approx_fast(rec[:], tden[:])
                    tnum = tmp.tile([P, CH], F32, tag="rec")
                    nc.scalar.activation(
                        tnum[:], pn[:], Act.Identity,
                        bias=scol[:, dti : dti + 1],
                    )
                    rat = tmp.tile([P, CH], F32, tag="rec")
                    nc.vector.tensor_tensor(rat[:], tnum[:], rec[:], Alu.mult)
                    nc.vector.scalar_tensor_tensor(
                        oPC[j][:, dti, :], rat[:], bv_s[:, dti : dti + 1],
                        sigQT[:, dti, ds(toffs[j], CH)], Alu.add, Alu.mult,
                    )
                    # export pair-t halves as soon as their d-tiles are
                    # done so the AllGather fires early (jitter slack
                    # for the output projection's pair wave)
                    if pos < 2 and dti % 2 == 1:
                        bv_view = b_in[pos].rearrange(
                            "(o p) t -> p o t", p=P
                        )
                        nc.sync.dma_start(
                            bv_view[:, dti - 1 : dti + 1, :],
                            oPC[j][:, dti - 1 : dti + 1, :],
                        )

            for i in range(2):
                nc.gpsimd.collective_compute(
                    "AllGather",
                    mybir.AluOpType.bypass,
                    replica_groups=PAIRS,
                    ins=[b_in[i].opt()],
                    outs=[S_out[i].opt()],
                )
                nc.gpsimd.dma_start(
                    S_sb[i][:],
                    S_out[i].rearrange("(o p) t -> p o t", p=P)[
                        :, ds((1 - par) * DHT, DHT), :
                    ],
                )

            # ---- output projection for own-t rows, full d ----
            # two waves of 4 t-tiles; within a wave all own-half (local
            # oPC) contractions run first so the PE keeps busy while the
            # pair exchange finishes, then the S_sb half completes them.
            for wave in range(2):
                tts = list(range(wave * 4, wave * 4 + 4))
                pys = {}
                for tt in tts:
                    own = oPC[tt // DHT]
                    tof = (tt % DHT) * P
                    for ec in range(2):
                        esl = slice(ec * CH, (ec + 1) * CH)
                        py = psum.tile(
                            [P, CH], F32, tag="ps", name=f"py{tt}_{ec}"
                        )
                        pys[(tt, ec)] = py
                        for k in range(4):
                            nc.tensor.matmul(
                                py[:], own[:, k, tof : tof + P],
                                wo_s[:, k, esl],
                                start=(k == 0), stop=False,
                            )
                for tt in tts:
                    ssb = S_sb[tt // DHT]
                    tof = (tt % DHT) * P
                    ysb = tmp.tile([P, D], BF, tag="ysb", bufs=2)
                    for ec in range(2):
                        esl = slice(ec * CH, (ec + 1) * CH)
                        py = pys[(tt, ec)]
                        for k in range(4):
                            nc.tensor.matmul(
                                py[:], ssb[:, k, tof : tof + P],
                                wo_s[:, 4 + k, esl],
                                start=False, stop=(k == 3),
                            )
                        nc.vector.tensor_tensor(
                            ysb[:, esl], py[:], bo_s[:, esl], Alu.add
                        )
                        # each 512-col half exports as two quarters on
                        # sync + scalar in parallel: halves the per-queue
                        # line count, so the final drain is ~0.9us
                        e0, e1 = ec * CH, ec * CH + CH // 2
                        nc.sync.dma_start(
                            y_v[:, tt, e0 : e0 + CH // 2],
                            ysb[:, e0 : e0 + CH // 2],
                        )
                        nc.scalar.dma_start(
                            y_v[:, tt, e1 : e1 + CH // 2],
                            ysb[:, e1 : e1 + CH // 2],
                        )

    nc.compile()
    return nc


def _get_nc():
    if "nc" not in _cache:
        _cache["nc"] = _build_nc()
    return _cache["nc"]


def kernel(x, dummy, Wq, bq, Wk, bk, Wv, bv, Wo, bo, wbias):
    import os

    x = np.asarray(x, np.float32)
    Wq = np.asarray(Wq, np.float32)
    Wk = np.asarray(Wk, np.float32)
    Wv = np.asarray(Wv, np.float32)
    Wo = np.asarray(Wo, np.float32)
    bq = np.asarray(bq, np.float32)
    bv = np.asarray(bv, np.float32)
    bo = np.asarray(bo, np.float32)
    wbias = np.asarray(wbias, np.float32)

    wqTf = np.ascontiguousarray(Wq.T)  # [d_in, e_out] fp32
    wkTf = np.ascontiguousarray(Wk.T)
    wvTf = np.ascontiguousarray(Wv.T)
    woTf = np.ascontiguousarray(Wo.T)  # rows = d
    # E^T = (exp(wbias) - 1)^T, scaled for fp8e4 (max |E*4096| ~ 160)
    e8 = np.clip(
        (np.exp(wbias).T.astype(np.float64) - 1.0) * ESC, -240.0, 240.0
    ).astype(F8E4)
    bob = np.ascontiguousarray(np.broadcast_to(bo, (P, D)))

    def sbuf_image(w):
        # [KT*P, F] -> [P, KT*F]: row p = concat over k of w[k*P+p, :]
        kt = w.shape[0] // P
        return np.ascontiguousarray(
            w.reshape(kt, P, -1).transpose(1, 0, 2).reshape(P, -1)
        )

    def f8(w, s):
        return np.clip(w * s, -240.0, 240.0).astype(F8E4)

    in_maps = []
    for c in range(8):
        b, p = c // 2, c % 2
        dlo, dhi = p * DH, (p + 1) * DH
        qlo, qhi = (1 - p) * DH, (2 - p) * DH
        woTp = np.concatenate([woTf[dlo:dhi], woTf[qlo:qhi]], axis=0)
        # E8 chunks pre-rotated by parity: chunk j covers t columns
        # (j*512 + p*1024) mod 2048, laid out as that chunk's SBUF image
        e8F = np.empty((TC * P, ST * CH), F8E4)
        for j in range(TC):
            w0 = (j * CH + p * TH) % T
            blk = e8[:, w0 : w0 + CH]  # [T, CH]
            e8F[j * P : (j + 1) * P] = (
                blk.reshape(ST, P, CH).transpose(1, 0, 2).reshape(P, -1)
            )
        xT = np.ascontiguousarray(x[b].T)
        in_maps.append(
            {
                "x8F": f8(xT, 1.0),
                "xbF": xT.astype(F16),
                "wqF": sbuf_image(f8(wqTf[:, dlo:dhi], QSC)),
                "wkF": sbuf_image(f8(wkTf[:, dlo:dhi], KSC)),
                "wvF": sbuf_image(wvTf[:, dlo:dhi].astype(F16)),
                "woF": sbuf_image(woTp.astype(F16)),
                "e8F": e8F,
                "bqc": np.ascontiguousarray(bq[dlo:dhi].reshape(DHT, P).T),
                "bvc": np.ascontiguousarray(bv[dlo:dhi].reshape(DHT, P).T),
                "bob": bob,
            }
        )

    from concourse.bass_utils import run_bass_kernel_spmd

    nc = _get_nc()
    trace = bool(os.environ.get("AFT_TRACE"))
    if not trace:
        os.environ["BASS_NEVER_TRACE"] = "1"
    res = run_bass_kernel_spmd(
        nc, in_maps, core_ids=list(range(8)), trace=trace
    )
    kernel._last_exec_ns = res.exec_time_ns
    kernel._last_result = res

    out = np.empty((B, T, D), np.float32)
    for c in range(8):
        b, p = c // 2, c % 2
        out[b, p * TH : (p + 1) * TH, :] = res.results[c]["y"].astype(
            np.float32
        )
    return out


# revision 32
# speedup vs baseline: 1.1743x; 1.1743x over previous
"""AFT-Full on 8 TRN2 cores, v10: fp8-DR K/Q, fp16 V, streamed PE.

Same d-split pair sharding as v3 (core c: batch c//2, parity c%2; each
core projects K/V/Q for its d-half, mixes over all t, pairs exchange
activation halves, each core output-projects its own t-half).

Numerics: the AFT ratio is a weighted average of V over all 2048
positions, so its RELATIVE error equals the per-entry relative noise of
both the V values and the eK weights — V and the S_K/S_u sums must stay
>= fp16 precision (fp8 there costs ~2.5% output error). K tolerates fp8
(its 1.4% noise lands once, total ~9e-3 vs the 2e-2 gate): it runs fp8
DoubleRow from host-quantized x8/wk8 (x2048, undone by the exp's input
scale). Q stays fp8 DR (sigmoid output); everything formerly bf16 is
fp16 (same PE speed, 8x less rounding noise — HW error == numpy-sim
error). u8/eK8 stay fp8e4 for the E-side mixing only (2% of magnitude).

Scheduling (all measured on HW):
- Every 16-bit matmul streams at ~215ns/512-col slot; fp8 DR doubles
  the contraction per slot. The PE runs >98% busy between the fixed
  ~10us framework preamble and a ~2us output drain.
- DMA queues: gpsimd ring ~170GB/s, scalar ~75, sync ~60; transfers
  move full 2-16KB per-partition lines. A dma_start push BLOCKS its
  engine until the queue drains to ~2 in flight, so the scalar engine
  (which must run the exp/copy evacuations) carries only 3 early
  pushes, and wo/bo load late into the x pool's freed space.
- s-tile group 1 of K runs st-major and V group 1 k-major with
  interleaved evacuation, so PSUM banks recycle one at a time instead
  of stalling the PE at group boundaries.
- The S_u adds pair-sum on the DVE (fp16) before an 8-deep Pool-engine
  chain (a 16-deep Pool chain at 1.16us/add throttled ub recycling and
  stalled Q by ~9us); eK8 = fp8(eKb) is a DVE tensor_copy.
- All four E8 chunks stay resident (no WAR restage), so the gpsimd
  ring reaches the AllGather triggers immediately; b_in halves export
  as soon as their d-tiles finish; sync stays clean of bulk traffic so
  the exchange fires the moment data is ready (out-projection wave 2
  overlaps the pair exchange).
- (rat + bv) * sigQ fused into one DVE scalar_tensor_tensor; scol's
  partition-reduce hides behind the first mixing unit's matmuls; y is
  written fp16 per 512-column half, alternating sync/scalar queues.

Bias identities as in v3: bk cancels in the ratio, bv is a post-ratio
add, bq fuses into the sigmoid, bo rides the output-projection PSUM
evacuation.
"""

import sys

if "/opt/trn_rl_repo" not in sys.path:
    sys.path.insert(0, "/opt/trn_rl_repo")

import numpy as np
import ml_dtypes

F16 = np.float16
F8E4 = ml_dtypes.float8_e4m3

B, T, D = 4, 2048, 1024
TH = T // 2   # own-t rows per core
DH = D // 2   # d-half
P = 128
CH = 512
KT = D // P    # 8 k-tiles (full-d contractions)
DHT = DH // P  # 4 d-tiles in my half
ST = T // P    # 16 s-tiles
TC = T // CH   # 4 t-chunks of the full sequence
ESC = 4096.0   # fp8 scale on E; cancels in the ratio
KSC = 2048.0   # fp8 scale on wk; undone by the exp's input scale
QSC = 64.0     # fp8 scale on wq; undone by the sigmoid's input scale
WARM = 14

_cache = {}


def _build_nc():
    from contextlib import ExitStack

    import concourse.mybir as mybir
    import concourse.tile as tile
    from concourse import bacc
    from concourse.bass import ds

    dt = mybir.dt
    BF = dt.float16
    F32 = dt.float32
    F8 = dt.float8e4
    Act = mybir.ActivationFunctionType
    Alu = mybir.AluOpType
    DR = mybir.MatmulPerfMode.DoubleRow
    PAIRS = [[0, 1], [2, 3], [4, 5], [6, 7]]

    nc = bacc.Bacc("TRN2")

    # weights arrive host-restaged as SBUF images (row p = that
    # partition's full free-axis line) so every DMA moves 4-8KB lines
    x8F = nc.dram_tensor("x8F", [D, T], F8, kind="ExternalInput")
    xbF = nc.dram_tensor("xbF", [D, T], BF, kind="ExternalInput")
    wqF = nc.dram_tensor("wqF", [P, KT * DH], F8, kind="ExternalInput")
    wkF = nc.dram_tensor("wkF", [P, KT * DH], F8, kind="ExternalInput")
    wvF = nc.dram_tensor("wvF", [P, KT * DH], BF, kind="ExternalInput")
    woF = nc.dram_tensor("woF", [P, KT * D], BF, kind="ExternalInput")
    # E8 chunks pre-rotated by this core's parity: row j*128+p = the
    # SBUF line of partition p for AFT chunk j
    e8F = nc.dram_tensor("e8F", [TC * P, ST * CH], F8, kind="ExternalInput")
    bqc = nc.dram_tensor("bqc", [P, DHT], F32, kind="ExternalInput")
    bvc = nc.dram_tensor("bvc", [P, DHT], F32, kind="ExternalInput")
    bob = nc.dram_tensor("bob", [P, D], F32, kind="ExternalInput")
    y = nc.dram_tensor("y", [TH, D], BF, kind="ExternalOutput")

    x8_v = x8F.rearrange("(o p) t -> p o t", p=P)
    xb_v = xbF.rearrange("(o p) t -> p o t", p=P)
    e8_v = e8F.rearrange("(j p) t -> p j t", p=P)
    y_v = y.rearrange("(o p) e -> p o e", p=P)

    with tile.TileContext(nc) as tc:
        with (
            tc.tile_pool(name="big", bufs=1) as big,
            tc.tile_pool(name="w", bufs=2) as wpool,
            tc.tile_pool(name="tmp", bufs=6) as tmp,
            tc.tile_pool(name="bias", bufs=1) as biasp,
            tc.tile_pool(name="ew", bufs=4) as ewpool,
            tc.tile_pool(name="sg", bufs=1) as sgpool,
            tc.tile_pool(name="dram", bufs=4, space="DRAM") as dram,
            tc.tile_pool(name="psum", bufs=8, space="PSUM") as psum,
            ExitStack() as late_ctx,
        ):
            pid = nc.partition_id()
            par = pid % 2

            eKb = big.tile([P, ST, DH], BF, tag="eKb")
            eK8 = big.tile([P, ST, DH], F8, tag="eK8")
            u8 = big.tile([P, ST, DH], F8, tag="u8")
            sigQT = big.tile([P, DHT, T], BF, tag="sigQT")
            accK = big.tile([P, DH], F32, tag="accK")
            accU = big.tile([P, DH], F32, tag="accU")
            # scol columns: 0..3 = 4096*S_u per d-tile, 4..7 = 4096*S_K
            scol = big.tile([P, 2 * DHT], F32, tag="scol")
            ones1 = big.tile([P, 1], F32, tag="ones1")
            # AFT output in rotated-t order: chunk j holds t columns
            # (j*512 + p*1024) mod 2048 .. +512; j=0,1 own-t, j=2,3 pair-t
            oPC = [
                big.tile([P, DHT, CH], BF, tag=f"oPC{j}", name=f"oPC{j}")
                for j in range(TC)
            ]
            # staged E8 chunks, all four alive (processing order 2,3,0,1)
            E8c = {}

            nc.vector.memset(accK[:], 0.0)
            nc.vector.memset(accU[:], 0.0)
            nc.vector.memset(ones1[:], ESC)

            with tc.tile_pool(name="x", bufs=1) as xpool:
                x8 = xpool.tile([P, KT, T], F8, tag="x8")
                xb = xpool.tile([P, KT, T], BF, tag="xb")
                wk_s = wpool.tile([P, KT, DH], F8, tag="w")
                wv_s = wpool.tile([P, KT, DH], BF, tag="w")

                # PE warm-up during the input-DMA wait
                warm = biasp.tile([P, CH], BF, tag="warm")
                nc.vector.memset(warm[:], 0.0)
                pwarm = psum.tile([P, CH], F32, tag="ps", name="pwarm")
                for _ in range(WARM):
                    nc.tensor.matmul(
                        pwarm[:], warm[:, :P], warm[:], start=True, stop=True
                    )

                # DMA scheduling facts (measured): per-queue rates are
                # ~170GB/s on gpsimd's software ring, ~75 on scalar's,
                # ~60 on sync's; a push BLOCKS its issuing engine until
                # the queue has slots (~2 in flight), so the scalar
                # engine — which must run the K evacuation exps from
                # ~20us — gets at most 3 early pushes. Everything moves
                # full rows (2-16KB lines).
                nc.scalar.dma_start(x8[:, 0, :], x8_v[:, 0, :])
                nc.scalar.dma_start(x8[:, 6, :], x8_v[:, 6, :])
                nc.scalar.dma_start(xb[:, 1, :TH], xb_v[:, 1, :TH])
                nc.sync.dma_start(x8[:, 2, :], x8_v[:, 2, :])
                nc.sync.dma_start(x8[:, 4, :], x8_v[:, 4, :])
                nc.gpsimd.dma_start(wk_s[:, :, :], wkF[:, :])
                nc.gpsimd.dma_start(x8[:, 1, :], x8_v[:, 1, :])
                nc.gpsimd.dma_start(x8[:, 3, :], x8_v[:, 3, :])
                nc.gpsimd.dma_start(x8[:, 5, :], x8_v[:, 5, :])
                nc.gpsimd.dma_start(x8[:, 7, :], x8_v[:, 7, :])
                nc.gpsimd.dma_start(xb[:, 0, :TH], xb_v[:, 0, :TH])
                nc.gpsimd.dma_start(wv_s[:, :4, :], wvF[:, : 4 * DH])
                nc.gpsimd.dma_start(wv_s[:, 4:, :], wvF[:, 4 * DH :])
                for k in [2, 3]:
                    nc.gpsimd.dma_start(xb[:, k, :TH], xb_v[:, k, :TH])
                for k in [4, 5, 6, 7]:
                    nc.sync.dma_start(xb[:, k, :TH], xb_v[:, k, :TH])
                # second halves all on gpsimd: V group 1 is k-major and
                # consumes them progressively from ~0.3 in
                for k in range(KT):
                    nc.gpsimd.dma_start(xb[:, k, TH:], xb_v[:, k, TH:])
                bq_s = biasp.tile([P, DHT], F32, tag="bq")
                nc.sync.dma_start(bq_s[:], bqc[:])
                bv_s = biasp.tile([P, DHT], F32, tag="bv")
                nc.sync.dma_start(bv_s[:], bvc[:])
                wq_s = wpool.tile([P, KT, DH], F8, tag="wq", bufs=1)
                nc.gpsimd.dma_start(wq_s[:, :, :], wqF[:, :])

                # E8 chunks (host pre-rotated by parity, static APs): all
                # FOUR buffers resident (no restage WAR), so the gpsimd
                # ring flows straight through to the collective triggers
                # — the first AllGather fires the moment its export
                # lands instead of queuing behind a gated restage.
                for j in [2, 3, 0, 1]:
                    ec = ewpool.tile([P, ST, CH], F8, tag="ewc", name=f"E8c{j}")
                    nc.gpsimd.dma_start(ec[:, :, :], e8_v[:, j, :])
                    E8c[j] = ec

                def k_evac(st):
                    nc.scalar.activation(
                        eKb[:, st, :], pks[st][:], Act.Exp, scale=1.0 / KSC
                    )
                    nc.vector.tensor_copy(eK8[:, st, :], eKb[:, st, :])
                    nc.gpsimd.tensor_tensor(
                        accK[:], accK[:], eKb[:, st, :], Alu.add
                    )

                # ---- K projection (d-half, fp8 DR) -> eKb/eK8/accK ----
                # group 0 contraction-major (overlaps the x8/wk8 DMA
                # ramp), group 1 st-major (PSUM banks recycle one at a
                # time through group 0's evacuations)
                pks = {}
                for st in range(8):
                    pks[st] = psum.tile([P, CH], F32, tag="ps", name=f"pk{st}")
                for kp in range(KT // 2):
                    ksl = slice(2 * kp, 2 * kp + 2)
                    for st in range(8):
                        nc.tensor.matmul(
                            pks[st][:],
                            x8[:, ksl, st * P : (st + 1) * P],
                            wk_s[:, ksl, :],
                            start=(kp == 0), stop=(kp == KT // 2 - 1),
                            perf_mode=DR,
                        )
                for st in range(8):
                    k_evac(st)
                for st in range(8, 16):
                    pks[st] = psum.tile([P, CH], F32, tag="ps", name=f"pk{st}")
                    for kp in range(KT // 2):
                        ksl = slice(2 * kp, 2 * kp + 2)
                        nc.tensor.matmul(
                            pks[st][:],
                            x8[:, ksl, st * P : (st + 1) * P],
                            wk_s[:, ksl, :],
                            start=(kp == 0), stop=(kp == KT // 2 - 1),
                            perf_mode=DR,
                        )
                    k_evac(st)

                # the Pool engine runs ~1.16us per [P,CH] add — a 16-deep
                # accU chain there throttles ub recycling and stalls the
                # PE into Q. Instead: DVE sums st-pairs in fp16 (fast,
                # 2x rate), gpsimd chains only the 8 pair sums.
                vodd = [None]

                def v_evac(st):
                    ub = tmp.tile([P, CH], BF, tag="ub", bufs=4)
                    nc.vector.tensor_tensor(
                        ub[:], eKb[:, st, :], pvs[st][:], Alu.mult
                    )
                    nc.scalar.copy(u8[:, st, :], ub[:])
                    if st % 2 == 0:
                        vodd[0] = ub
                    else:
                        up = tmp.tile([P, CH], BF, tag="up", bufs=3)
                        nc.vector.tensor_tensor(
                            up[:], vodd[0][:], ub[:], Alu.add
                        )
                        nc.gpsimd.tensor_tensor(
                            accU[:], accU[:], up[:], Alu.add
                        )

                # ---- V projection (d-half, bf16) -> u8/accU ----
                pvs = {}
                for st in range(8):
                    pvs[st] = psum.tile([P, CH], F32, tag="ps", name=f"pv{st}")
                for k in range(KT):
                    for st in range(8):
                        nc.tensor.matmul(
                            pvs[st][:],
                            xb[:, k, st * P : (st + 1) * P],
                            wv_s[:, k, :],
                            start=(k == 0), stop=(k == KT - 1),
                        )
                for st in range(8):
                    v_evac(st)
                # group 1 k-major (the xb second halves stream in per
                # k-tile during group 0), evacuations interleaved into
                # the last k pass so the banks recycle one at a time
                for st in range(8, 16):
                    pvs[st] = psum.tile([P, CH], F32, tag="ps", name=f"pv{st}")
                for k in range(KT):
                    for st in range(8, 16):
                        nc.tensor.matmul(
                            pvs[st][:],
                            xb[:, k, st * P : (st + 1) * P],
                            wv_s[:, k, :],
                            start=(k == 0), stop=(k == KT - 1),
                        )
                        if k == KT - 1:
                            v_evac(st)

                # ---- Q^T projection (d-half e, ALL t), fp8 DR -> sigQT ----
                for et in range(DHT):
                    esl = slice(et * P, (et + 1) * P)
                    for c in range(TC):
                        tsl = slice(c * CH, (c + 1) * CH)
                        pq = psum.tile([P, CH], F32, tag="ps")
                        for kp in range(KT // 2):
                            ksl = slice(2 * kp, 2 * kp + 2)
                            nc.tensor.matmul(
                                pq[:], wq_s[:, ksl, esl], x8[:, ksl, tsl],
                                start=(kp == 0), stop=(kp == KT // 2 - 1),
                                perf_mode=DR,
                            )
                        nc.scalar.activation(
                            sigQT[:, et, tsl], pq[:], Act.Sigmoid,
                            bias=bq_s[:, et : et + 1], scale=1.0 / QSC,
                        )

                # (scol is reduced inside the first mixing unit below, so
                # the PE never head-of-line waits on the acc chains)

            # wo + bo load into the space the x pool just freed, on the
            # scalar queue (ring free after the xb halves; keeps sync
            # CLEAN so the b_in exports below never queue behind a 2MB
            # transfer — that jitter stalled the out-projection ~10us)
            wopool = late_ctx.enter_context(tc.tile_pool(name="wop", bufs=1))
            wo_s = wopool.tile([P, KT, D], BF, tag="wo", name="wo_s")
            nc.scalar.dma_start(wo_s[:, :4, :], woF[:, : 4 * D])
            nc.scalar.dma_start(wo_s[:, 4:, :], woF[:, 4 * D :])
            bo_s = wopool.tile([P, D], F32, tag="bo")
            nc.scalar.dma_start(bo_s[:], bob[:])

            # ---- AFT mixing, fp8 DoubleRow over chunk pairs; pair-t
            # chunks (2,3) first so the exchange hides under (0,1) ----
            b_in = [None, None]
            S_out = [None, None]
            S_sb = [None, None]
            for i in range(2):
                b_in[i] = dram.tile([DH, CH], BF, name=f"bin{i}")
                S_out[i] = dram.tile([2 * DH, CH], BF, name=f"sout{i}")
                S_sb[i] = sgpool.tile(
                    [P, DHT, CH], BF, tag=f"ssb{i}", name=f"ssb{i}"
                )

            toffs = {}
            for j in range(TC):
                toffs[j] = nc.s_assert_within(
                    (j * CH + par * TH) & (T - 1), 0, T - CH,
                    skip_runtime_assert=True,
                )

            for pos, j in enumerate([2, 3, 0, 1]):
                for dti in range(DHT):
                    dsl = slice(dti * P, (dti + 1) * P)
                    pn = psum.tile([P, CH], F32, tag="ps", name="pn")
                    pd = psum.tile([P, CH], F32, tag="ps", name="pd")
                    for sp in range(ST // 2):
                        ksl = slice(2 * sp, 2 * sp + 2)
                        nc.tensor.matmul(
                            pn[:], u8[:, ksl, dsl], E8c[j][:, ksl, :],
                            start=(sp == 0), stop=(sp == ST // 2 - 1),
                            perf_mode=DR,
                        )
                    for sp in range(ST // 2):
                        ksl = slice(2 * sp, 2 * sp + 2)
                        nc.tensor.matmul(
                            pd[:], eK8[:, ksl, dsl], E8c[j][:, ksl, :],
                            start=(sp == 0), stop=(sp == ST // 2 - 1),
                            perf_mode=DR,
                        )
                    if pos == 0 and dti == 0:
                        # partition-reduce the column sums (scol = 4096*S)
                        # behind the first unit's matmuls: the tden/tnum
                        # evacuations below are the first scol readers
                        pscol = psum.tile(
                            [P, 2 * DHT], F32, tag="ps", name="pscol"
                        )
                        for dtc in range(DHT):
                            dcl = slice(dtc * P, (dtc + 1) * P)
                            nc.tensor.matmul(
                                pscol[:, dtc : dtc + 1], accU[:, dcl],
                                ones1[:], start=True, stop=True,
                            )
                            nc.tensor.matmul(
                                pscol[:, DHT + dtc : DHT + dtc + 1],
                                accK[:, dcl], ones1[:],
                                start=True, stop=True,
                            )
                        nc.scalar.copy(scol[:], pscol[:])
                    tden = tmp.tile([P, CH], F32, tag="rec")
                    nc.scalar.activation(
                        tden[:], pd[:], Act.Identity,
                        bias=scol[:, DHT + dti : DHT + dti + 1],
                    )
                    rec = tmp.tile([P, CH], F32, tag="rec")
                    nc.vector.reciprocal_approx_fast(rec[:], tden[:])
                    tnum = tmp.tile([P, CH], F32, tag="rec")
                    nc.scalar.activation(
                        tnum[:], pn[:], Act.Identity,
                        bias=scol[:, dti : dti + 1],
                    )
                    rat = tmp.tile([P, CH], F32, tag="rec")
                    nc.vector.tensor_tensor(rat[:], tnum[:], rec[:], Alu.mult)
                    nc.vector.scalar_tensor_tensor(
                        oPC[j][:, dti, :], rat[:], bv_s[:, dti : dti + 1],
                        sigQT[:, dti, ds(toffs[j], CH)], Alu.add, Alu.mult,
                    )
                    # export pair-t halves as soon as their d-tiles are
                    # done so the AllGather fires early (jitter slack
                    # for the output projection's pair wave)
                    if pos < 2 and dti % 2 == 1:
                        bv_view = b_in[pos].rearrange(
                            "(o p) t -> p o t", p=P
                        )
                        nc.sync.dma_start(
                            bv_view[:, dti - 1 : dti + 1, :],
                            oPC[j][:, dti - 1 : dti + 1, :],
                        )

            for i in range(2):
                nc.gpsimd.collective_compute(
                    "AllGather",
                    mybir.AluOpType.bypass,
                    replica_groups=PAIRS,
                    ins=[b_in[i].opt()],
                    outs=[S_out[i].opt()],
                )
                nc.gpsimd.dma_start(
                    S_sb[i][:],
                    S_out[i].rearrange("(o p) t -> p o t", p=P)[
                        :, ds((1 - par) * DHT, DHT), :
                    ],
                )

            # ---- output projection for own-t rows, full d ----
            # two waves of 4 t-tiles; within a wave all own-half (local
            # oPC) contractions run first so the PE keeps busy while the
            # pair exchange finishes, then the S_sb half completes them.
            for wave in range(2):
                tts = list(range(wave * 4, wave * 4 + 4))
                pys = {}
                for tt in tts:
                    own = oPC[tt // DHT]
                    tof = (tt % DHT) * P
                    for ec in range(2):
                        esl = slice(ec * CH, (ec + 1) * CH)
                        py = psum.tile(
                            [P, CH], F32, tag="ps", name=f"py{tt}_{ec}"
                        )
                        pys[(tt, ec)] = py
                        for k in range(4):
                            nc.tensor.matmul(
                                py[:], own[:, k, tof : tof + P],
                                wo_s[:, k, esl],
                                start=(k == 0), stop=False,
                            )
                for tt in tts:
                    ssb = S_sb[tt // DHT]
                    tof = (tt % DHT) * P
                    ysb = tmp.tile([P, D], BF, tag="ysb", bufs=2)
                    for ec in range(2):
                        esl = slice(ec * CH, (ec + 1) * CH)
                        py = pys[(tt, ec)]
                        for k in range(4):
                            nc.tensor.matmul(
                                py[:], ssb[:, k, tof : tof + P],
                                wo_s[:, 4 + k, esl],
                                start=False, stop=(k == 3),
                            )
                        nc.vector.tensor_tensor(
                            ysb[:, esl], py[:], bo_s[:, esl], Alu.add
                        )
                        # each 512-col half exports as two quarters on
                        # sync + scalar in parallel: halves the per-queue
                        # line count, so the final drain is ~0.9us
                        e0, e1 = ec * CH, ec * CH + CH // 2
                        nc.sync.dma_start(
                            y_v[:, tt, e0 : e0 + CH // 2],
                            ysb[:, e0 : e0 + CH // 2],
                        )
                        nc.scalar.dma_start(
                            y_v[:, tt, e1 : e1 + CH // 2],
                            ysb[:, e1 : e1 + CH // 2],
                        )

    nc.compile()
    return nc


def _get_nc():
    if "nc" not in _cache:
        _cache["nc"] = _build_nc()
    return _cache["nc"]


def kernel(x, dummy, Wq, bq, Wk, bk, Wv, bv, Wo, bo, wbias):
    import os

    x = np.asarray(x, np.float32)
    Wq = np.asarray(Wq, np.float32)
    Wk = np.asarray(Wk, np.float32)
    Wv = np.asarray(Wv, np.float32)
    Wo = np.asarray(Wo, np.float32)
    bq = np.asarray(bq, np.float32)
    bv = np.asarray(bv, np.float32)
    bo = np.asarray(bo, np.float32)
    wbias = np.asarray(wbias, np.float32)

    wqTf = np.ascontiguousarray(Wq.T)  # [d_in, e_out] fp32
    wkTf = np.ascontiguousarray(Wk.T)
    wvTf = np.ascontiguousarray(Wv.T)
    woTf = np.ascontiguousarray(Wo.T)  # rows = d
    # E^T = (exp(wbias) - 1)^T, scaled for fp8e4 (max |E*4096| ~ 160)
    e8 = np.clip(
        (np.exp(wbias).T.astype(np.float64) - 1.0) * ESC, -240.0, 240.0
    ).astype(F8E4)
    bob = np.ascontiguousarray(np.broadcast_to(bo, (P, D)))

    def sbuf_image(w):
        # [KT*P, F] -> [P, KT*F]: row p = concat over k of w[k*P+p, :]
        kt = w.shape[0] // P
        return np.ascontiguousarray(
            w.reshape(kt, P, -1).transpose(1, 0, 2).reshape(P, -1)
        )

    def f8(w, s):
        return np.clip(w * s, -240.0, 240.0).astype(F8E4)

    in_maps = []
    for c in range(8):
        b, p = c // 2, c % 2
        dlo, dhi = p * DH, (p + 1) * DH
        qlo, qhi = (1 - p) * DH, (2 - p) * DH
        woTp = np.concatenate([woTf[dlo:dhi], woTf[qlo:qhi]], axis=0)
        # E8 chunks pre-rotated by parity: chunk j covers t columns
        # (j*512 + p*1024) mod 2048, laid out as that chunk's SBUF image
        e8F = np.empty((TC * P, ST * CH), F8E4)
        for j in range(TC):
            w0 = (j * CH + p * TH) % T
            blk = e8[:, w0 : w0 + CH]  # [T, CH]
            e8F[j * P : (j + 1) * P] = (
                blk.reshape(ST, P, CH).transpose(1, 0, 2).reshape(P, -1)
            )
        xT = np.ascontiguousarray(x[b].T)
        in_maps.append(
            {
                "x8F": f8(xT, 1.0),
                "xbF": xT.astype(F16),
                "wqF": sbuf_image(f8(wqTf[:, dlo:dhi], QSC)),
                "wkF": sbuf_image(f8(wkTf[:, dlo:dhi], KSC)),
                "wvF": sbuf_image(wvTf[:, dlo:dhi].astype(F16)),
                "woF": sbuf_image(woTp.astype(F16)),
                "e8F": e8F,
                "bqc": np.ascontiguousarray(bq[dlo:dhi].reshape(DHT, P).T),
                "bvc": np.ascontiguousarray(bv[dlo:dhi].reshape(DHT, P).T),
                "bob": bob,
            }
        )

    from concourse.bass_utils import run_bass_kernel_spmd

    nc = _get_nc()
    trace = bool(os.environ.get("AFT_TRACE"))
    if not trace:
        os.environ["BASS_NEVER_TRACE"] = "1"
    res = run_bass_kernel_spmd(
        nc, in_maps, core_ids=list(range(8)), trace=trace
    )
    kernel._last_exec_ns = res.exec_time_ns
    kernel._last_result = res

    out = np.empty((B, T, D), np.float32)
    for c in range(8):
        b, p = c // 2, c % 2
        out[b, p * TH : (p + 1) * TH, :] = res.results[c]["y"].astype(
            np.float32
        )
    return out


# revision 33
# speedup vs baseline: 1.1862x; 1.0102x over previous
"""AFT-Full on 8 TRN2 cores, v10: fp8-DR K/Q, fp16 V, streamed PE.

Same d-split pair sharding as v3 (core c: batch c//2, parity c%2; each
core projects K/V/Q for its d-half, mixes over all t, pairs exchange
activation halves, each core output-projects its own t-half).

Numerics: the AFT ratio is a weighted average of V over all 2048
positions, so its RELATIVE error equals the per-entry relative noise of
both the V values and the eK weights — V and the S_K/S_u sums must stay
>= fp16 precision (fp8 there costs ~2.5% output error). K tolerates fp8
(its 1.4% noise lands once, total ~9e-3 vs the 2e-2 gate): it runs fp8
DoubleRow from host-quantized x8/wk8 (x2048, undone by the exp's input
scale). Q stays fp8 DR (sigmoid output); everything formerly bf16 is
fp16 (same PE speed, 8x less rounding noise — HW error == numpy-sim
error). u8/eK8 stay fp8e4 for the E-side mixing only (2% of magnitude).

Scheduling (all measured on HW):
- Every 16-bit matmul streams at ~215ns/512-col slot; fp8 DR doubles
  the contraction per slot. The PE runs >98% busy between the fixed
  ~10us framework preamble and a ~2us output drain.
- DMA queues: gpsimd ring ~170GB/s, scalar ~75, sync ~60; transfers
  move full 2-16KB per-partition lines. A dma_start push BLOCKS its
  engine until the queue drains to ~2 in flight, so the scalar engine
  (which must run the exp/copy evacuations) carries only 3 early
  pushes, and wo/bo load late into the x pool's freed space.
- s-tile group 1 of K runs st-major and V group 1 k-major with
  interleaved evacuation, so PSUM banks recycle one at a time instead
  of stalling the PE at group boundaries.
- The S_u adds pair-sum on the DVE (fp16) before an 8-deep Pool-engine
  chain (a 16-deep Pool chain at 1.16us/add throttled ub recycling and
  stalled Q by ~9us); eK8 = fp8(eKb) is a DVE tensor_copy.
- All four E8 chunks stay resident (no WAR restage), so the gpsimd
  ring reaches the AllGather triggers immediately; b_in halves export
  as soon as their d-tiles finish; sync stays clean of bulk traffic so
  the exchange fires the moment data is ready (out-projection wave 2
  overlaps the pair exchange).
- (rat + bv) * sigQ fused into one DVE scalar_tensor_tensor; scol's
  partition-reduce hides behind the first mixing unit's matmuls; y is
  written fp16 per 512-column half, alternating sync/scalar queues.

Bias identities as in v3: bk cancels in the ratio, bv is a post-ratio
add, bq fuses into the sigmoid, bo rides the output-projection PSUM
evacuation.
"""

import sys

if "/opt/trn_rl_repo" not in sys.path:
    sys.path.insert(0, "/opt/trn_rl_repo")

import numpy as np
import ml_dtypes

F16 = np.float16
F8E4 = ml_dtypes.float8_e4m3

B, T, D = 4, 2048, 1024
TH = T // 2   # own-t rows per core
DH = D // 2   # d-half
P = 128
CH = 512
KT = D // P    # 8 k-tiles (full-d contractions)
DHT = DH // P  # 4 d-tiles in my half
ST = T // P    # 16 s-tiles
TC = T // CH   # 4 t-chunks of the full sequence
ESC = 4096.0   # fp8 scale on E; cancels in the ratio
KSC = 2048.0   # fp8 scale on wk; undone by the exp's input scale
QSC = 64.0     # fp8 scale on wq; undone by the sigmoid's input scale
WARM = 14

_cache = {}


def _build_nc():
    from contextlib import ExitStack

    import concourse.mybir as mybir
    import concourse.tile as tile
    from concourse import bacc
    from concourse.bass import ds

    dt = mybir.dt
    BF = dt.float16
    F32 = dt.float32
    F8 = dt.float8e4
    Act = mybir.ActivationFunctionType
    Alu = mybir.AluOpType
    DR = mybir.MatmulPerfMode.DoubleRow
    PAIRS = [[0, 1], [2, 3], [4, 5], [6, 7]]

    nc = bacc.Bacc("TRN2")

    # weights arrive host-restaged as SBUF images (row p = that
    # partition's full free-axis line) so every DMA moves 4-8KB lines
    x8F = nc.dram_tensor("x8F", [D, T], F8, kind="ExternalInput")
    xbF = nc.dram_tensor("xbF", [D, T], BF, kind="ExternalInput")
    wqF = nc.dram_tensor("wqF", [P, KT * DH], F8, kind="ExternalInput")
    wkF = nc.dram_tensor("wkF", [P, KT * DH], F8, kind="ExternalInput")
    wvF = nc.dram_tensor("wvF", [P, KT * DH], BF, kind="ExternalInput")
    woF = nc.dram_tensor("woF", [P, KT * D], BF, kind="ExternalInput")
    # E8 chunks pre-rotated by this core's parity: row j*128+p = the
    # SBUF line of partition p for AFT chunk j
    e8F = nc.dram_tensor("e8F", [TC * P, ST * CH], F8, kind="ExternalInput")
    bqc = nc.dram_tensor("bqc", [P, DHT], F32, kind="ExternalInput")
    bvc = nc.dram_tensor("bvc", [P, DHT], F32, kind="ExternalInput")
    bob = nc.dram_tensor("bob", [P, D], F32, kind="ExternalInput")
    y = nc.dram_tensor("y", [TH, D], BF, kind="ExternalOutput")

    x8_v = x8F.rearrange("(o p) t -> p o t", p=P)
    xb_v = xbF.rearrange("(o p) t -> p o t", p=P)
    e8_v = e8F.rearrange("(j p) t -> p j t", p=P)
    y_v = y.rearrange("(o p) e -> p o e", p=P)

    with tile.TileContext(nc) as tc:
        with (
            tc.tile_pool(name="big", bufs=1) as big,
            tc.tile_pool(name="w", bufs=2) as wpool,
            tc.tile_pool(name="tmp", bufs=6) as tmp,
            tc.tile_pool(name="bias", bufs=1) as biasp,
            tc.tile_pool(name="ew", bufs=4) as ewpool,
            tc.tile_pool(name="sg", bufs=1) as sgpool,
            tc.tile_pool(name="dram", bufs=4, space="DRAM") as dram,
            tc.tile_pool(name="psum", bufs=8, space="PSUM") as psum,
            ExitStack() as late_ctx,
        ):
            pid = nc.partition_id()
            par = pid % 2

            eKb = big.tile([P, ST, DH], BF, tag="eKb")
            eK8 = big.tile([P, ST, DH], F8, tag="eK8")
            u8 = big.tile([P, ST, DH], F8, tag="u8")
            sigQT = big.tile([P, DHT, T], BF, tag="sigQT")
            accK = big.tile([P, DH], F32, tag="accK")
            accU = big.tile([P, DH], F32, tag="accU")
            # scol columns: 0..3 = 4096*S_u per d-tile, 4..7 = 4096*S_K
            scol = big.tile([P, 2 * DHT], F32, tag="scol")
            ones1 = big.tile([P, 1], F32, tag="ones1")
            # AFT output in rotated-t order: chunk j holds t columns
            # (j*512 + p*1024) mod 2048 .. +512; j=0,1 own-t, j=2,3 pair-t
            oPC = [
                big.tile([P, DHT, CH], BF, tag=f"oPC{j}", name=f"oPC{j}")
                for j in range(TC)
            ]
            # staged E8 chunks, all four alive (processing order 2,3,0,1)
            E8c = {}

            nc.vector.memset(accK[:], 0.0)
            nc.vector.memset(accU[:], 0.0)
            nc.vector.memset(ones1[:], ESC)

            with tc.tile_pool(name="x", bufs=1) as xpool:
                x8 = xpool.tile([P, KT, T], F8, tag="x8")
                xb = xpool.tile([P, KT, T], BF, tag="xb")
                wk_s = wpool.tile([P, KT, DH], F8, tag="w")
                wv_s = wpool.tile([P, KT, DH], BF, tag="w")

                # PE warm-up during the input-DMA wait
                warm = biasp.tile([P, CH], BF, tag="warm")
                nc.vector.memset(warm[:], 0.0)
                pwarm = psum.tile([P, CH], F32, tag="ps", name="pwarm")
                for _ in range(WARM):
                    nc.tensor.matmul(
                        pwarm[:], warm[:, :P], warm[:], start=True, stop=True
                    )

                # DMA scheduling facts (measured): per-queue rates are
                # ~170GB/s on gpsimd's software ring, ~75 on scalar's,
                # ~60 on sync's; a push BLOCKS its issuing engine until
                # the queue has slots (~2 in flight), so the scalar
                # engine — which must run the K evacuation exps from
                # ~20us — gets at most 3 early pushes. Everything moves
                # full rows (2-16KB lines).
                nc.scalar.dma_start(x8[:, 0, :], x8_v[:, 0, :])
                nc.scalar.dma_start(x8[:, 6, :], x8_v[:, 6, :])
                nc.scalar.dma_start(xb[:, 1, :TH], xb_v[:, 1, :TH])
                nc.sync.dma_start(x8[:, 2, :], x8_v[:, 2, :])
                nc.sync.dma_start(x8[:, 4, :], x8_v[:, 4, :])
                nc.gpsimd.dma_start(wk_s[:, :, :], wkF[:, :])
                nc.gpsimd.dma_start(x8[:, 1, :], x8_v[:, 1, :])
                nc.gpsimd.dma_start(x8[:, 3, :], x8_v[:, 3, :])
                nc.gpsimd.dma_start(x8[:, 5, :], x8_v[:, 5, :])
                nc.gpsimd.dma_start(x8[:, 7, :], x8_v[:, 7, :])
                nc.gpsimd.dma_start(xb[:, 0, :TH], xb_v[:, 0, :TH])
                nc.gpsimd.dma_start(wv_s[:, :4, :], wvF[:, : 4 * DH])
                nc.gpsimd.dma_start(wv_s[:, 4:, :], wvF[:, 4 * DH :])
                for k in [2, 3]:
                    nc.gpsimd.dma_start(xb[:, k, :TH], xb_v[:, k, :TH])
                for k in [4, 5, 6, 7]:
                    nc.sync.dma_start(xb[:, k, :TH], xb_v[:, k, :TH])
                # second halves all on gpsimd: V group 1 is k-major and
                # consumes them progressively from ~0.3 in
                for k in range(KT):
                    nc.gpsimd.dma_start(xb[:, k, TH:], xb_v[:, k, TH:])
                bq_s = biasp.tile([P, DHT], F32, tag="bq")
                nc.sync.dma_start(bq_s[:], bqc[:])
                bv_s = biasp.tile([P, DHT], F32, tag="bv")
                nc.sync.dma_start(bv_s[:], bvc[:])
                wq_s = wpool.tile([P, KT, DH], F8, tag="wq", bufs=1)
                nc.gpsimd.dma_start(wq_s[:, :, :], wqF[:, :])

                # E8 chunks (host pre-rotated by parity, static APs): all
                # FOUR buffers resident (no restage WAR), so the gpsimd
                # ring flows straight through to the collective triggers
                # — the first AllGather fires the moment its export
                # lands instead of queuing behind a gated restage.
                for j in [2, 3, 0, 1]:
                    ec = ewpool.tile([P, ST, CH], F8, tag="ewc", name=f"E8c{j}")
                    nc.gpsimd.dma_start(ec[:, :, :], e8_v[:, j, :])
                    E8c[j] = ec

                def k_evac(st):
                    nc.scalar.activation(
                        eKb[:, st, :], pks[st][:], Act.Exp, scale=1.0 / KSC
                    )
                    nc.vector.tensor_copy(eK8[:, st, :], eKb[:, st, :])
                    nc.gpsimd.tensor_tensor(
                        accK[:], accK[:], eKb[:, st, :], Alu.add
                    )

                # ---- K projection (d-half, fp8 DR) -> eKb/eK8/accK ----
                # group 0 contraction-major (overlaps the x8/wk8 DMA
                # ramp), group 1 st-major (PSUM banks recycle one at a
                # time through group 0's evacuations)
                pks = {}
                for st in range(8):
                    pks[st] = psum.tile([P, CH], F32, tag="ps", name=f"pk{st}")
                for kp in range(KT // 2):
                    ksl = slice(2 * kp, 2 * kp + 2)
                    for st in range(8):
                        nc.tensor.matmul(
                            pks[st][:],
                            x8[:, ksl, st * P : (st + 1) * P],
                            wk_s[:, ksl, :],
                            start=(kp == 0), stop=(kp == KT // 2 - 1),
                            perf_mode=DR,
                        )
                for st in range(8):
                    k_evac(st)
                for st in range(8, 16):
                    pks[st] = psum.tile([P, CH], F32, tag="ps", name=f"pk{st}")
                    for kp in range(KT // 2):
                        ksl = slice(2 * kp, 2 * kp + 2)
                        nc.tensor.matmul(
                            pks[st][:],
                            x8[:, ksl, st * P : (st + 1) * P],
                            wk_s[:, ksl, :],
                            start=(kp == 0), stop=(kp == KT // 2 - 1),
                            perf_mode=DR,
                        )
                    k_evac(st)

                # the Pool engine runs ~1.16us per [P,CH] add — a 16-deep
                # accU chain there throttles ub recycling and stalls the
                # PE into Q. Instead: DVE sums st-pairs in fp16 (fast,
                # 2x rate), gpsimd chains only the 8 pair sums.
                vodd = [None]

                def v_evac(st):
                    ub = tmp.tile([P, CH], BF, tag="ub", bufs=4)
                    nc.vector.tensor_tensor(
                        ub[:], eKb[:, st, :], pvs[st][:], Alu.mult
                    )
                    nc.scalar.copy(u8[:, st, :], ub[:])
                    if st % 2 == 0:
                        vodd[0] = ub
                    else:
                        up = tmp.tile([P, CH], BF, tag="up", bufs=3)
                        nc.vector.tensor_tensor(
                            up[:], vodd[0][:], ub[:], Alu.add
                        )
                        nc.gpsimd.tensor_tensor(
                            accU[:], accU[:], up[:], Alu.add
                        )

                # ---- V projection (d-half, bf16) -> u8/accU ----
                pvs = {}
                for st in range(8):
                    pvs[st] = psum.tile([P, CH], F32, tag="ps", name=f"pv{st}")
                for k in range(KT):
                    for st in range(8):
                        nc.tensor.matmul(
                            pvs[st][:],
                            xb[:, k, st * P : (st + 1) * P],
                            wv_s[:, k, :],
                            start=(k == 0), stop=(k == KT - 1),
                        )
                for st in range(8):
                    v_evac(st)
                # group 1 k-major (the xb second halves stream in per
                # k-tile during group 0), evacuations interleaved into
                # the last k pass so the banks recycle one at a time
                for st in range(8, 16):
                    pvs[st] = psum.tile([P, CH], F32, tag="ps", name=f"pv{st}")
                for k in range(KT):
                    for st in range(8, 16):
                        nc.tensor.matmul(
                            pvs[st][:],
                            xb[:, k, st * P : (st + 1) * P],
                            wv_s[:, k, :],
                            start=(k == 0), stop=(k == KT - 1),
                        )
                        if k == KT - 1:
                            v_evac(st)

                # ---- Q^T projection (d-half e, ALL t), fp8 DR -> sigQT ----
                for et in range(DHT):
                    esl = slice(et * P, (et + 1) * P)
                    for c in range(TC):
                        tsl = slice(c * CH, (c + 1) * CH)
                        pq = psum.tile([P, CH], F32, tag="ps")
                        for kp in range(KT // 2):
                            ksl = slice(2 * kp, 2 * kp + 2)
                            nc.tensor.matmul(
                                pq[:], wq_s[:, ksl, esl], x8[:, ksl, tsl],
                                start=(kp == 0), stop=(kp == KT // 2 - 1),
                                perf_mode=DR,
                            )
                        nc.scalar.activation(
                            sigQT[:, et, tsl], pq[:], Act.Sigmoid,
                            bias=bq_s[:, et : et + 1], scale=1.0 / QSC,
                        )

                # (scol is reduced inside the first mixing unit below, so
                # the PE never head-of-line waits on the acc chains)

            # wo + bo load into the space the x pool just freed, on the
            # scalar queue (ring free after the xb halves; keeps sync
            # CLEAN so the b_in exports below never queue behind a 2MB
            # transfer — that jitter stalled the out-projection ~10us)
            wopool = late_ctx.enter_context(tc.tile_pool(name="wop", bufs=1))
            wo_s = wopool.tile([P, KT, D], BF, tag="wo", name="wo_s")
            nc.scalar.dma_start(wo_s[:, :4, :], woF[:, : 4 * D])
            nc.scalar.dma_start(wo_s[:, 4:, :], woF[:, 4 * D :])
            bo_s = wopool.tile([P, D], F32, tag="bo")
            nc.scalar.dma_start(bo_s[:], bob[:])

            # ---- AFT mixing, fp8 DoubleRow over chunk pairs; pair-t
            # chunks (2,3) first so the exchange hides under (0,1) ----
            b_in = [None, None]
            S_out = [None, None]
            S_sb = [None, None]
            for i in range(2):
                b_in[i] = dram.tile([DH, CH], BF, name=f"bin{i}")
                S_out[i] = dram.tile([2 * DH, CH], BF, name=f"sout{i}")
                S_sb[i] = sgpool.tile(
                    [P, DHT, CH], BF, tag=f"ssb{i}", name=f"ssb{i}"
                )

            toffs = {}
            for j in range(TC):
                toffs[j] = nc.s_assert_within(
                    (j * CH + par * TH) & (T - 1), 0, T - CH,
                    skip_runtime_assert=True,
                )

            for pos, j in enumerate([2, 3, 0, 1]):
                for dti in range(DHT):
                    dsl = slice(dti * P, (dti + 1) * P)
                    pn = psum.tile([P, CH], F32, tag="ps", name="pn")
                    pd = psum.tile([P, CH], F32, tag="ps", name="pd")
                    for sp in range(ST // 2):
                        ksl = slice(2 * sp, 2 * sp + 2)
                        nc.tensor.matmul(
                            pn[:], u8[:, ksl, dsl], E8c[j][:, ksl, :],
                            start=(sp == 0), stop=(sp == ST // 2 - 1),
                            perf_mode=DR,
                        )
                    for sp in range(ST // 2):
                        ksl = slice(2 * sp, 2 * sp + 2)
                        nc.tensor.matmul(
                            pd[:], eK8[:, ksl, dsl], E8c[j][:, ksl, :],
                            start=(sp == 0), stop=(sp == ST // 2 - 1),
                            perf_mode=DR,
                        )
                    if pos == 0 and dti == 0:
                        # partition-reduce the column sums (scol = 4096*S)
                        # behind the first unit's matmuls: the tden/tnum
                        # evacuations below are the first scol readers
                        pscol = psum.tile(
                            [P, 2 * DHT], F32, tag="ps", name="pscol"
                        )
                        for dtc in range(DHT):
                            dcl = slice(dtc * P, (dtc + 1) * P)
                            nc.tensor.matmul(
                                pscol[:, dtc : dtc + 1], accU[:, dcl],
                                ones1[:], start=True, stop=True,
                            )
                            nc.tensor.matmul(
                                pscol[:, DHT + dtc : DHT + dtc + 1],
                                accK[:, dcl], ones1[:],
                                start=True, stop=True,
                            )
                        nc.scalar.copy(scol[:], pscol[:])
                    tden = tmp.tile([P, CH], F32, tag="rec")
                    nc.scalar.activation(
                        tden[:], pd[:], Act.Identity,
                        bias=scol[:, DHT + dti : DHT + dti + 1],
                    )
                    rec = tmp.tile([P, CH], F32, tag="rec")
                    nc.vector.reciprocal_approx_fast(rec[:], tden[:])
                    tnum = tmp.tile([P, CH], F32, tag="rec")
                    nc.scalar.activation(
                        tnum[:], pn[:], Act.Identity,
                        bias=scol[:, dti : dti + 1],
                    )
                    rat = tmp.tile([P, CH], F32, tag="rec")
                    nc.vector.tensor_tensor(rat[:], tnum[:], rec[:], Alu.mult)
                    nc.vector.scalar_tensor_tensor(
                        oPC[j][:, dti, :], rat[:], bv_s[:, dti : dti + 1],
                        sigQT[:, dti, ds(toffs[j], CH)], Alu.add, Alu.mult,
                    )
                    # export pair-t halves as soon as their d-tiles are
                    # done so the AllGather fires early (jitter slack
                    # for the output projection's pair wave)
                    if pos < 2 and dti % 2 == 1:
                        bv_view = b_in[pos].rearrange(
                            "(o p) t -> p o t", p=P
                        )
                        nc.sync.dma_start(
                            bv_view[:, dti - 1 : dti + 1, :],
                            oPC[j][:, dti - 1 : dti + 1, :],
                        )

            for i in range(2):
                nc.gpsimd.collective_compute(
                    "AllGather",
                    mybir.AluOpType.bypass,
                    replica_groups=PAIRS,
                    ins=[b_in[i].opt()],
                    outs=[S_out[i].opt()],
                )
                nc.gpsimd.dma_start(
                    S_sb[i][:],
                    S_out[i].rearrange("(o p) t -> p o t", p=P)[
                        :, ds((1 - par) * DHT, DHT), :
                    ],
                )

            # ---- output projection for own-t rows, full d ----
            # two waves of 4 t-tiles; within a wave all own-half (local
            # oPC) contractions run first so the PE keeps busy while the
            # pair exchange finishes, then the S_sb half completes them.
            for wave in range(2):
                tts = list(range(wave * 4, wave * 4 + 4))
                pys = {}
                for tt in tts:
                    own = oPC[tt // DHT]
                    tof = (tt % DHT) * P
                    for ec in range(2):
                        esl = slice(ec * CH, (ec + 1) * CH)
                        py = psum.tile(
                            [P, CH], F32, tag="ps", name=f"py{tt}_{ec}"
                        )
                        pys[(tt, ec)] = py
                        for k in range(4):
                            nc.tensor.matmul(
                                py[:], own[:, k, tof : tof + P],
                                wo_s[:, k, esl],
                                start=(k == 0), stop=False,
                            )
                for tt in tts:
                    ssb = S_sb[tt // DHT]
                    tof = (tt % DHT) * P
                    ysb = tmp.tile([P, D], BF, tag="ysb", bufs=2)
                    for ec in range(2):
                        esl = slice(ec * CH, (ec + 1) * CH)
                        py = pys[(tt, ec)]
                        for k in range(4):
                            nc.tensor.matmul(
                                py[:], ssb[:, k, tof : tof + P],
                                wo_s[:, 4 + k, esl],
                                start=False, stop=(k == 3),
                            )
                        # each 512-col half exports as two quarters on
                        # sync + scalar in parallel: halves the per-queue
                        # line count, so the final drain is ~0.9us. The
                        # very last tile evacuates per-quarter so each
                        # DMA fires ~0.35us sooner.
                        e0, e1 = ec * CH, ec * CH + CH // 2
                        if tt == 7:
                            nc.vector.tensor_tensor(
                                ysb[:, e0 : e0 + CH // 2],
                                py[:, : CH // 2],
                                bo_s[:, e0 : e0 + CH // 2], Alu.add,
                            )
                            nc.sync.dma_start(
                                y_v[:, tt, e0 : e0 + CH // 2],
                                ysb[:, e0 : e0 + CH // 2],
                            )
                            nc.vector.tensor_tensor(
                                ysb[:, e1 : e1 + CH // 2],
                                py[:, CH // 2 :],
                                bo_s[:, e1 : e1 + CH // 2], Alu.add,
                            )
                            nc.scalar.dma_start(
                                y_v[:, tt, e1 : e1 + CH // 2],
                                ysb[:, e1 : e1 + CH // 2],
                            )
                        else:
                            nc.vector.tensor_tensor(
                                ysb[:, esl], py[:], bo_s[:, esl], Alu.add
                            )
                            nc.sync.dma_start(
                                y_v[:, tt, e0 : e0 + CH // 2],
                                ysb[:, e0 : e0 + CH // 2],
                            )
                            nc.scalar.dma_start(
                                y_v[:, tt, e1 : e1 + CH // 2],
                                ysb[:, e1 : e1 + CH // 2],
                            )

    nc.compile()
    return nc


def _get_nc():
    if "nc" not in _cache:
        _cache["nc"] = _build_nc()
    return _cache["nc"]


def kernel(x, dummy, Wq, bq, Wk, bk, Wv, bv, Wo, bo, wbias):
    import os

    x = np.asarray(x, np.float32)
    Wq = np.asarray(Wq, np.float32)
    Wk = np.asarray(Wk, np.float32)
    Wv = np.asarray(Wv, np.float32)
    Wo = np.asarray(Wo, np.float32)
    bq = np.asarray(bq, np.float32)
    bv = np.asarray(bv, np.float32)
    bo = np.asarray(bo, np.float32)
    wbias = np.asarray(wbias, np.float32)

    wqTf = np.ascontiguousarray(Wq.T)  # [d_in, e_out] fp32
    wkTf = np.ascontiguousarray(Wk.T)
    wvTf = np.ascontiguousarray(Wv.T)
    woTf = np.ascontiguousarray(Wo.T)  # rows = d
    # E^T = (exp(wbias) - 1)^T, scaled for fp8e4 (max |E*4096| ~ 160)
    e8 = np.clip(
        (np.exp(wbias).T.astype(np.float64) - 1.0) * ESC, -240.0, 240.0
    ).astype(F8E4)
    bob = np.ascontiguousarray(np.broadcast_to(bo, (P, D)))

    def sbuf_image(w):
        # [KT*P, F] -> [P, KT*F]: row p = concat over k of w[k*P+p, :]
        kt = w.shape[0] // P
        return np.ascontiguousarray(
            w.reshape(kt, P, -1).transpose(1, 0, 2).reshape(P, -1)
        )

    def f8(w, s):
        return np.clip(w * s, -240.0, 240.0).astype(F8E4)

    in_maps = []
    for c in range(8):
        b, p = c // 2, c % 2
        dlo, dhi = p * DH, (p + 1) * DH
        qlo, qhi = (1 - p) * DH, (2 - p) * DH
        woTp = np.concatenate([woTf[dlo:dhi], woTf[qlo:qhi]], axis=0)
        # E8 chunks pre-rotated by parity: chunk j covers t columns
        # (j*512 + p*1024) mod 2048, laid out as that chunk's SBUF image
        e8F = np.empty((TC * P, ST * CH), F8E4)
        for j in range(TC):
            w0 = (j * CH + p * TH) % T
            blk = e8[:, w0 : w0 + CH]  # [T, CH]
            e8F[j * P : (j + 1) * P] = (
                blk.reshape(ST, P, CH).transpose(1, 0, 2).reshape(P, -1)
            )
        xT = np.ascontiguousarray(x[b].T)
        in_maps.append(
            {
                "x8F": f8(xT, 1.0),
                "xbF": xT.astype(F16),
                "wqF": sbuf_image(f8(wqTf[:, dlo:dhi], QSC)),
                "wkF": sbuf_image(f8(wkTf[:, dlo:dhi], KSC)),
                "wvF": sbuf_image(wvTf[:, dlo:dhi].astype(F16)),
                "woF": sbuf_image(woTp.astype(F16)),
                "e8F": e8F,
                "bqc": np.ascontiguousarray(bq[dlo:dhi].reshape(DHT, P).T),
                "bvc": np.ascontiguousarray(bv[dlo:dhi].reshape(DHT, P).T),
                "bob": bob,
            }
        )

    from concourse.bass_utils import run_bass_kernel_spmd

    nc = _get_nc()
    trace = bool(os.environ.get("AFT_TRACE"))
    if not trace:
        os.environ["BASS_NEVER_TRACE"] = "1"
    res = run_bass_kernel_spmd(
        nc, in_maps, core_ids=list(range(8)), trace=trace
    )
    kernel._last_exec_ns = res.exec_time_ns
    kernel._last_result = res

    out = np.empty((B, T, D), np.float32)
    for c in range(8):
        b, p = c // 2, c % 2
        out[b, p * TH : (p + 1) * TH, :] = res.results[c]["y"].astype(
            np.float32
        )
    return out
